# revision 1
# baseline (speedup 1.0000x reference)
import numpy as np
import jax
import jax.numpy as jnp

# Problem dims (hardcoded per spec: nn_AttentionAggregator2_46033459479180)
N, K, D, E, H, O = 16384, 32, 256, 64, 64, 256
M = 8  # NeuronCores; data-parallel shard over node dim N


def _compute(x, neibs, edge_emb, mask, att_w1, att_w2, att2_w1, att2_w2, fcx_w, fcn_w):
    n = x.shape[0]
    neibs_cat = jnp.concatenate([neibs, edge_emb], axis=1)            # [n*K, D+E]
    # Fold att2_w2 into the node side: scores = tanh(pre1) . z with
    # z = x_att @ att2_w2 — avoids materializing neib_att @ att2_w2.T.
    pre1 = jnp.tanh(neibs_cat @ att2_w1.T)                            # [n*K, H]
    x_att = jnp.tanh(x @ att_w1.T) @ att_w2.T                         # [n, H]
    z = x_att @ att2_w2                                               # [n, H]
    pre1 = pre1.reshape(n, K, H)
    ws = jnp.einsum("nkh,nh->nk", pre1, z) / np.float32(np.sqrt(H))
    ws = ws + (-9999999.0) * mask
    ws = jax.nn.softmax(ws, axis=1)                                   # [n, K]
    agg_neib = neibs_cat.reshape(n, K, D + E)
    agg = jnp.einsum("nk,nkd->nd", ws, agg_neib)                      # [n, D+E]
    out = jnp.concatenate([x @ fcx_w.T, agg @ fcn_w.T], axis=1)       # [n, 2*O]
    return jax.nn.relu(out)


def kernel(x, neibs, edge_emb, mask, att_w1, att_w2, att2_w1, att2_w2, fcx_w, fcn_w):
    x = np.asarray(x, dtype=np.float32)
    neibs = np.asarray(neibs, dtype=np.float32)
    edge_emb = np.asarray(edge_emb, dtype=np.float32)
    mask = np.asarray(mask, dtype=np.float32)
    weights = [np.asarray(w, dtype=np.float32)
               for w in (att_w1, att_w2, att2_w1, att2_w2, fcx_w, fcn_w)]

    devs = jax.devices()
    if len(devs) >= M:
        # Shard rows of x/neibs/edge_emb/mask across the 8 cores; replicate weights.
        nl = N // M
        xs = x.reshape(M, nl, D)
        ns = neibs.reshape(M, nl * K, D)
        es = edge_emb.reshape(M, nl * K, E)
        ms = mask.reshape(M, nl, K)
        rep = lambda w: np.broadcast_to(w, (M,) + w.shape)
        try:
            out = jax.pmap(_compute, devices=devs[:M])(
                xs, ns, es, ms, *[rep(w) for w in weights])
            return np.asarray(out).reshape(N, 2 * O).astype(np.float32)
        except Exception:
            pass
    # Fallback: single-device (still correct)
    out = jax.jit(_compute)(x, neibs, edge_emb, mask, *weights)
    return np.asarray(out).astype(np.float32)

